# revision 1
# baseline (speedup 1.0000x reference)
"""AdjacentAttention on 8 TRN2 NeuronCores.

Strategy (all shapes hardcoded for B=1, N=10000, A=32, D=256, H=4, DH=64):

Host:
  - kv projection commutes with the neighbor gather, so the device computes a
    kv table (x @ Wkv, bf16) once and gathers *projected* rows, 32x less
    matmul work than the reference's gather-then-project.
  - ~50% of neighbors are masked out.  The host compacts each node's
    neighbor list to its valid entries, sorts nodes by degree, and deals
    them into 10 degree-homogeneous tile groups of 1024 (128 nodes x 8
    cores), so each tile only gathers/computes its group-max degree a_t
    instead of A=32.
  - x is passed pre-transposed (and bf16) so no on-device transposes are
    needed for the matmuls; attention-scale is folded into Wq.

Device (SPMD, identical program per core, no collectives):
  Phase A: kv table = x @ Wkv for all 10112 padded rows -> HBM scratch (bf16).
  Phase B: q tiles for this core's 1280 permuted nodes.
  Phase C: per node-tile: dma_gather of kv rows for (node, valid-neighbor)
    pairs + a null slot; q.k on DVE (bf16 2x) + halving trees; softmax with
    f32 denominators; attn-weighted v-sum; PE transpose + out-projection.
"""

import os

import numpy as np
import ml_dtypes

import bass_rust
import concourse.bacc as bacc
import concourse.tile as tile
from concourse import mybir
from concourse.bass_utils import run_bass_kernel_spmd

BF = ml_dtypes.bfloat16

N, A, D, H, DH = 10000, 32, 256, 4, 64
NCORES, P, NT = 8, 128, 10
GROUP = NCORES * P            # 1024 nodes per tile-group
NPAD = NT * GROUP             # 10240
KV_TILES = (N + P - 1) // P   # 79
NKV = KV_TILES * P            # 10112 padded kv-table rows
HD = H * DH                   # 256
KVW = 2 * HD                  # 512 (k|v row width)

LAST_EXEC_NS = None


def _build(a_ts):
    nc = bacc.Bacc("TRN2", target_bir_lowering=False, num_swdge_queues=2)
    bf = mybir.dt.bfloat16
    f32 = mybir.dt.float32
    mult = mybir.AluOpType.mult
    add = mybir.AluOpType.add

    aas = [a + 1 for a in a_ts]
    idxcols = 8 * sum(a_ts)
    mcols = sum(aas)

    xT = nc.declare_dram_parameter("xT", [P, 2, NKV], bf, isOutput=False)
    xpT = nc.declare_dram_parameter("xpT", [P, 2, NT * P], bf, isOutput=False)
    wq = nc.declare_dram_parameter("wq", [P, 2, HD], bf, isOutput=False)
    wkv = nc.declare_dram_parameter("wkv", [P, 2, KVW], bf, isOutput=False)
    wo = nc.declare_dram_parameter("wo", [P, 2, D], bf, isOutput=False)
    bo_p = nc.declare_dram_parameter("bo", [1, D], bf, isOutput=False)
    nullkv = nc.declare_dram_parameter("nullkv", [P, KVW], bf, isOutput=False)
    ident_p = nc.declare_dram_parameter("ident", [P, P], bf, isOutput=False)
    idxs_p = nc.declare_dram_parameter("idxs", [P, idxcols], mybir.dt.int16, isOutput=False)
    masks_p = nc.declare_dram_parameter("masks", [P, mcols], f32, isOutput=False)
    out_p = nc.declare_dram_parameter("out", [NT * P, D], f32, isOutput=True)

    kv_dram = nc.dram_tensor("kv_scratch", [NKV, KVW], bf)
    warm_dram = nc.dram_tensor("warm_scratch", [NKV, KVW], bf)

    with tile.TileContext(nc) as tc:
        with (
            tc.tile_pool(name="singles", bufs=1) as singles,
            tc.tile_pool(name="kvstage", bufs=4) as kvstage,
            tc.tile_pool(name="work", bufs=2) as work,
            tc.tile_pool(name="small", bufs=4) as small,
            tc.tile_pool(name="psA", bufs=2, space="PSUM") as psA,
            tc.tile_pool(name="psT", bufs=2, space="PSUM") as psT,
            tc.tile_pool(name="psF", bufs=2, space="PSUM") as psF,
        ):
            # ---------- constants ----------
            wq_sb = singles.tile([P, 2, HD], bf)
            nc.sync.dma_start(out=wq_sb[:], in_=wq[:])
            wkv_sb = singles.tile([P, 2, KVW], bf)
            nc.sync.dma_start(out=wkv_sb[:], in_=wkv[:])
            wo_sb = singles.tile([P, 2, D], bf)
            nc.sync.dma_start(out=wo_sb[:], in_=wo[:])
            bo_sb = singles.tile([1, D], bf)
            nc.sync.dma_start(out=bo_sb[:], in_=bo_p[:])
            nullkv_sb = singles.tile([P, KVW], bf)
            nc.sync.dma_start(out=nullkv_sb[:], in_=nullkv[:])
            ident_sb = singles.tile([P, P], bf)
            nc.sync.dma_start(out=ident_sb[:], in_=ident_p[:])
            idx_sb = singles.tile([P, idxcols], mybir.dt.int16)
            idx_dma = nc.sync.dma_start(out=idx_sb[:], in_=idxs_p[:])
            mask_sb = singles.tile([P, mcols], f32)
            nc.sync.dma_start(out=mask_sb[:], in_=masks_p[:])
            ones1 = singles.tile([1, P], bf)
            nc.vector.memset(ones1[:], 1.0)

            # ---------- phase A: kv table ----------
            kv_writes = []
            with tc.tile_pool(name="big", bufs=1) as big:
                x_sb = big.tile([P, 2, NKV], bf)
                nc.sync.dma_start(out=x_sb[:], in_=xT[:])
                i = 0
                gidx = 0
                while i < KV_TILES:
                    nb = min(2, KV_TILES - i)
                    ps = psA.tile([P, 2, KVW], f32, space="PSUM", tag="psA")
                    for j in range(nb):
                        nc.tensor.matmul(
                            out=ps[:, j, :],
                            lhsT=x_sb[:, 0, (i + j) * P:(i + j + 1) * P],
                            rhs=wkv_sb[:, 0, :], start=True, stop=False)
                        nc.tensor.matmul(
                            out=ps[:, j, :],
                            lhsT=x_sb[:, 1, (i + j) * P:(i + j + 1) * P],
                            rhs=wkv_sb[:, 1, :], start=False, stop=True)
                    st = kvstage.tile([P, 2, KVW], bf, tag="kvstage")
                    if gidx % 2 == 0:
                        nc.scalar.copy(out=st[:, 0:nb, :], in_=ps[:, 0:nb, :])
                    else:
                        nc.vector.tensor_copy(out=st[:, 0:nb, :], in_=ps[:, 0:nb, :])
                    dst = (kv_dram[i * P:(i + nb) * P, :]
                           .rearrange("(j p) c -> p j c", j=nb))
                    kv_writes.append(
                        nc.sync.dma_start(out=dst, in_=st[:, 0:nb, :]))
                    i += nb
                    gidx += 1

            kvgp_cm = tc.tile_pool(name="kvgp", bufs=5)
            kvgp = kvgp_cm.__enter__()

            # warmup: force the Q7 dma_gather library load + SWDGE path
            # setup during phase A (first real gather otherwise pays ~8us).
            warm = small.tile([P, 1, KVW], bf, tag="warm")
            for q in range(2):
                gw = nc.gpsimd.dma_gather(
                    warm[:], warm_dram[:], idx_sb[:, 0:8],
                    num_idxs=P, num_idxs_reg=P, elem_size=KVW,
                    single_packet=False, queue_num=q)
                bass_rust.add_dep_helper(gw.ins, idx_dma.ins,
                                         reason="warmup gather reads idx blob")

            # ---------- phase B: q tiles ----------
            xp_sb = singles.tile([P, 2, NT * P], bf)
            nc.sync.dma_start(out=xp_sb[:], in_=xpT[:])
            q_sb = singles.tile([P, NT, HD], bf)
            for t in range(NT):
                psq = psF.tile([P, HD], f32, space="PSUM", tag="psF")
                nc.tensor.matmul(
                    out=psq[:], lhsT=xp_sb[:, 0, t * P:(t + 1) * P],
                    rhs=wq_sb[:, 0, :], start=True, stop=False)
                nc.tensor.matmul(
                    out=psq[:], lhsT=xp_sb[:, 1, t * P:(t + 1) * P],
                    rhs=wq_sb[:, 1, :], start=False, stop=True)
                nc.scalar.copy(out=q_sb[:, t, :], in_=psq[:])

            # ---------- phase C: attention per tile ----------
            io = 0
            mo = 0
            CH = 8
            for t in range(len(a_ts)):
                a = a_ts[t]
                aa = a + 1
                kv_g = kvgp.tile([P, aa, KVW], bf, tag="kvg")
                nc.scalar.copy(out=kv_g[:, 0, :], in_=nullkv_sb[:])
                # chunk column cuts; first tile ramps up so the DVE pipeline
                # fills with minimum latency
                if t == 0:
                    ramp = [2, 4, 8]
                else:
                    ramp = []
                cuts = []
                c = 0
                ri = 0
                while c < a:
                    step = ramp[ri] if ri < len(ramp) else CH
                    ri += 1
                    c = min(c + step, a)
                    if c < a:
                        cuts.append(c)
                chunklist = list(zip([0] + cuts, cuts + [a]))
                ci = 0
                for c0, c1 in chunklist:
                    gi = nc.gpsimd.dma_gather(
                        kv_g[:, 1 + c0:1 + c1, :], kv_dram[:],
                        idx_sb[:, io + 8 * c0:io + 8 * c1],
                        num_idxs=P * (c1 - c0), num_idxs_reg=P * (c1 - c0),
                        elem_size=KVW, single_packet=False,
                        queue_num=(t + ci) % 2)
                    # Tile's auto-dep tracking misses dma_gather's *input*
                    # APs (idx tile + DRAM source); add edges explicitly.
                    bass_rust.add_dep_helper(gi.ins, idx_dma.ins,
                                             reason="gather reads idx blob")
                    for kw in kv_writes:
                        bass_rust.add_dep_helper(gi.ins, kw.ins,
                                                 reason="gather reads kv table")
                    ci += 1

                k4 = kv_g[:, :, 0:HD].rearrange("p a (h d) -> p a h d", d=DH)
                # process the q.k mul + dh-tree per gather-chunk so DVE
                # starts before the full tile lands
                bounds = [0] + [1 + c for c in cuts] + [aa]
                for b0, b1 in zip(bounds[:-1], bounds[1:]):
                    kc = k4[:, b0:b1]
                    qb = (q_sb[:, t:t + 1, :]
                          .rearrange("p o (h d) -> p o h d", d=DH)
                          .broadcast_to([P, b1 - b0, H, DH]))
                    nc.vector.tensor_tensor(out=kc, in0=kc, in1=qb, op=mult)
                    w = DH
                    while w > 1:
                        h2 = w // 2
                        nc.vector.tensor_tensor(
                            out=kc[:, :, :, 0:h2], in0=kc[:, :, :, 0:h2],
                            in1=kc[:, :, :, h2:w], op=add)
                        w = h2

                sim = kv_g[:, :, 0:HD:DH]          # [P, aa, H] strided
                exp_s = small.tile([P, aa, H], f32, tag="exp")
                nc.scalar.activation(
                    out=exp_s[:], in_=sim,
                    func=mybir.ActivationFunctionType.Exp)
                mb = (mask_sb[:, mo:mo + aa]
                      .rearrange("p (a o) -> p a o", o=1)
                      .broadcast_to([P, aa, H]))
                nc.vector.tensor_tensor(out=exp_s[:], in0=exp_s[:], in1=mb, op=mult)
                denom = small.tile([P, H], f32, tag="denom")
                nc.vector.tensor_reduce(
                    out=denom[:], in_=exp_s[:].rearrange("p a h -> p h a"),
                    axis=mybir.AxisListType.X, op=add)
                recip = small.tile([P, H], f32, tag="recip")
                nc.vector.reciprocal(out=recip[:], in_=denom[:])
                attn_b = small.tile([P, aa, H], f32, tag="attn")
                rb = (recip[:].rearrange("p (o h) -> p o h", o=1)
                      .broadcast_to([P, aa, H]))
                nc.vector.tensor_tensor(out=attn_b[:], in0=exp_s[:], in1=rb, op=mult)
                attn_x = work.tile([P, aa, H, DH], bf, tag="attnx")
                axin = (attn_b[:].rearrange("p a (h o) -> p a h o", o=1)
                        .broadcast_to([P, aa, H, DH]))
                nc.scalar.copy(out=attn_x[:], in_=axin)

                v4 = kv_g[:, :, HD:KVW].rearrange("p a (h d) -> p a h d", d=DH)
                nc.vector.tensor_tensor(out=v4, in0=v4, in1=attn_x[:], op=mult)
                w = aa
                while w > 1:
                    h2 = w // 2
                    nc.vector.tensor_tensor(
                        out=v4[:, 0:h2], in0=v4[:, 0:h2],
                        in1=v4[:, h2:2 * h2], op=add)
                    if w % 2 == 1:
                        nc.vector.tensor_tensor(
                            out=v4[:, 0:1], in0=v4[:, 0:1],
                            in1=v4[:, w - 1:w], op=add)
                    w = h2

                out_attn = kv_g[:, 0, HD:KVW]      # [P, 256] bf16
                outT = work.tile([P, 2, P], bf, tag="outT")
                for j in range(2):
                    pst = psT.tile([P, P], bf, space="PSUM", tag="psT")
                    nc.tensor.transpose(
                        out=pst[:], in_=out_attn[:, j * P:(j + 1) * P],
                        identity=ident_sb[:])
                    nc.scalar.copy(out=outT[:, j, :], in_=pst[:])

                psf = psF.tile([P, D], f32, space="PSUM", tag="psF")
                nc.tensor.matmul(out=psf[:], lhsT=ones1[0:1, :], rhs=bo_sb[0:1, :],
                                 start=True, stop=False)
                nc.tensor.matmul(out=psf[:], lhsT=outT[:, 0, :], rhs=wo_sb[:, 0, :],
                                 start=False, stop=False)
                nc.tensor.matmul(out=psf[:], lhsT=outT[:, 1, :], rhs=wo_sb[:, 1, :],
                                 start=False, stop=True)
                outf = small.tile([P, D], f32, tag="outf")
                nc.scalar.copy(out=outf[:], in_=psf[:])
                nc.sync.dma_start(out=out_p[t * P:(t + 1) * P, :], in_=outf[:])

                io += 8 * a
                mo += aa
            kvgp_cm.__exit__(None, None, None)

    nc.finalize()
    return nc


def _prep(x, adj, msk, Wq, Wkv, Wo, bo, null_k, null_v):
    """All host-side numpy prep. Returns (a_ts, in_maps, order)."""
    deg = msk.sum(1).astype(np.int64)
    order = np.concatenate([
        np.full(NPAD - N, -1, dtype=np.int64),
        np.argsort(deg, kind="stable"),
    ])

    a_by_group = []
    for g in range(NT):
        grp = order[g * GROUP:(g + 1) * GROUP]
        real = grp[grp >= 0]
        mx = int(deg[real].max()) if real.size else 0
        a_by_group.append(max(mx, 1))
    # emission order: smallest group first (fast pipeline fill), then
    # descending sizes (fat tiles mid-pipeline), ending with small tiles
    # (short drain tail).
    group_order = [0] + list(range(NT - 1, 0, -1))
    a_ts = [a_by_group[g] for g in group_order]

    # compact each node's neighbor list: valid entries first
    sortcols = np.argsort(~msk, axis=1, kind="stable")
    comp = np.take_along_axis(adj, sortcols, axis=1).astype(np.int16)

    scale = DH ** -0.5
    xpad = np.zeros((NKV, D), np.float32)
    xpad[:N] = x
    xT_h = np.ascontiguousarray(
        xpad.T.reshape(2, P, NKV).transpose(1, 0, 2)).astype(BF)
    wq_h = np.ascontiguousarray(
        (Wq * scale).reshape(2, P, HD).transpose(1, 0, 2)).astype(BF)
    wkv_h = np.ascontiguousarray(
        Wkv.reshape(2, P, KVW).transpose(1, 0, 2)).astype(BF)
    wo_h = np.ascontiguousarray(
        Wo.reshape(2, P, D).transpose(1, 0, 2)).astype(BF)
    bo_h = bo.reshape(1, D).astype(BF)
    nullkv_h = np.tile(
        np.concatenate([null_k.reshape(-1), null_v.reshape(-1)]).reshape(1, KVW),
        (P, 1)).astype(BF)
    ident_h = np.eye(P, dtype=np.float32).astype(BF)

    in_maps = []
    for c in range(NCORES):
        xp = np.zeros((NT * P, D), np.float32)
        flats = []
        mblocks = []
        for t, g in enumerate(group_order):
            a = a_ts[t]
            nodes = order[g * GROUP + c * P: g * GROUP + (c + 1) * P]
            nn = np.maximum(nodes, 0)
            xp[t * P:(t + 1) * P][nodes >= 0] = x[nodes[nodes >= 0]]
            valid = (np.arange(a)[None, :] < deg[nn][:, None]) & (nodes >= 0)[:, None]
            blk = np.where(valid, comp[nn, :a], 0).astype(np.int16)  # [128, a]
            flats.append(blk.T.reshape(-1))                          # i = col*128+p
            m = np.zeros((P, 1 + a), np.float32)
            m[:, 0] = 1.0
            m[:, 1:] = valid
            mblocks.append(m)
        flat = np.concatenate(flats)
        idx_h = np.ascontiguousarray(
            np.tile(flat.reshape(-1, 16).T, (8, 1))).astype(np.int16)
        mask_h = np.ascontiguousarray(np.concatenate(mblocks, axis=1))
        xpT_h = np.ascontiguousarray(
            xp.T.reshape(2, P, NT * P).transpose(1, 0, 2)).astype(BF)
        in_maps.append({
            "xT": xT_h, "xpT": xpT_h, "wq": wq_h, "wkv": wkv_h, "wo": wo_h,
            "bo": bo_h, "nullkv": nullkv_h, "ident": ident_h,
            "idxs": idx_h, "masks": mask_h,
        })
    return a_ts, in_maps, order


def kernel(x, adj_kv_indices, mask, Wq, Wkv, Wo, bo, null_k, null_v):
    global LAST_EXEC_NS
    x = np.asarray(x, dtype=np.float32)[0]
    adj = np.asarray(adj_kv_indices)[0].astype(np.int64)
    msk = np.asarray(mask)[0].astype(bool)
    Wq = np.asarray(Wq, np.float32)
    Wkv = np.asarray(Wkv, np.float32)
    Wo = np.asarray(Wo, np.float32)
    bo = np.asarray(bo, np.float32)
    null_k = np.asarray(null_k, np.float32)
    null_v = np.asarray(null_v, np.float32)

    a_ts, in_maps, order = _prep(x, adj, msk, Wq, Wkv, Wo, bo, null_k, null_v)
    nc = _build(tuple(a_ts))
    res = run_bass_kernel_spmd(
        nc, in_maps, core_ids=list(range(NCORES)),
        trace=bool(os.environ.get("KERNEL_TRACE")))
    LAST_EXEC_NS = res.exec_time_ns

    group_order = [0] + list(range(NT - 1, 0, -1))
    out_full = np.zeros((N, D), np.float32)
    for c in range(NCORES):
        o = np.asarray(res.results[c]["out"])
        for t, g in enumerate(group_order):
            nodes = order[g * GROUP + c * P: g * GROUP + (c + 1) * P]
            sel = nodes >= 0
            out_full[nodes[sel]] = o[t * P:(t + 1) * P][sel]
    return out_full.reshape(1, N, D)



# revision 10
# speedup vs baseline: 1.0014x; 1.0014x over previous
"""AdjacentAttention on 8 TRN2 NeuronCores.

Strategy (all shapes hardcoded for B=1, N=10000, A=32, D=256, H=4, DH=64):

Host:
  - kv projection commutes with the neighbor gather: the device computes a
    kv table (x @ Wkv, bf16) once and gathers *projected* rows.
  - ~50% of neighbors are masked out.  The host compacts each node's
    neighbor list to its valid entries, sorts nodes by degree, and deals
    them into 10 degree-homogeneous tile groups of 1024 (128 nodes x 8
    cores), so each tile only gathers/computes its group-max degree a_t.
  - Each node's valid neighbors are sorted ASCENDING by source row, so a
    gather chunk of low slot-columns only references low kv-table rows.
    The host computes, per gather chunk, the exact highest kv-write group
    it depends on -- gathers start while phase A is still writing.
  - The v half of Wkv's columns (and null_v, and Wo's rows) are permuted
    from (h, dh) to (dh, h) order so the attention-weighted v multiply can
    broadcast attn over the *middle* axis (DVE supports stride-0 middle
    broadcast, not inner), removing the big attn broadcast copy.
  - x is passed pre-transposed (and bf16) chunk-major so phase A streams
    it; attention-scale is folded into Wq.

Device (SPMD, identical program per core, no collectives):
  Phase B: q tiles for this core's 1280 permuted nodes (first, it's small).
  Phase A: kv table = x @ Wkv for all 10112 padded rows -> HBM scratch,
    streamed in x-chunks, 2-tile write groups.
  Phase C: per node-tile: dma_gather of kv rows for (node, valid-neighbor)
    pairs (8-slot chunks, each dep'd on the exact kv-write prefix it
    needs) + a null slot; q.k on DVE (bf16 2x) + dh halving tree; exp on
    ACT; w = mask*exp; denom reduce + reciprocal; w-weighted v tree-sum;
    normalize the 256-wide sum (not the aa*4 weights); PE transpose +
    out-projection.
"""

import os

import numpy as np
import ml_dtypes

import bass_rust
import concourse.bacc as bacc
import concourse.tile as tile
from concourse import mybir
from concourse.bass_utils import run_bass_kernel_spmd

BF = ml_dtypes.bfloat16

N, A, D, H, DH = 10000, 32, 256, 4, 64
NCORES, P, NT = 8, 128, 10
GROUP = NCORES * P            # 1024 nodes per tile-group
NPAD = NT * GROUP             # 10240
KV_TILES = (N + P - 1) // P   # 79
NKV = KV_TILES * P            # 10112 padded kv-table rows
HD = H * DH                   # 256
KVW = 2 * HD                  # 512 (k|v row width)
XCH = 10                      # x chunk / kv write group size in 128-row tiles
NWG = (KV_TILES + XCH - 1) // XCH  # 8 kv write groups

LAST_EXEC_NS = None


def _chunk_cuts(a, first):
    """Column cuts for gather chunks; first tile ramps up for pipe fill."""
    ramp = [4, 8] if first else []
    cuts = []
    c = 0
    ri = 0
    while c < a:
        step = ramp[ri] if ri < len(ramp) else 8
        ri += 1
        c = min(c + step, a)
        if c < a:
            cuts.append(c)
    return list(zip([0] + cuts, cuts + [a]))


def _build(a_ts, dep_groups):
    """a_ts: per-tile slot count.  dep_groups: per-tile tuple of kv-write
    group index (10-tile groups) each gather chunk depends on.  A gather
    gets edges to ALL write groups up to its dep (completion order across
    DMA queues is not FIFO)."""
    nc = bacc.Bacc("TRN2", target_bir_lowering=False, num_swdge_queues=2)
    bf = mybir.dt.bfloat16
    f32 = mybir.dt.float32
    mult = mybir.AluOpType.mult
    add = mybir.AluOpType.add

    aas = [a + 1 for a in a_ts]
    idxcols = 8 * sum(a_ts)
    mcols = sum(aas)

    xT = nc.declare_dram_parameter("xT", [P, KV_TILES, 2, P], bf, isOutput=False)
    xpT = nc.declare_dram_parameter("xpT", [P, 2, NT * P], bf, isOutput=False)
    wq = nc.declare_dram_parameter("wq", [P, 2, HD], bf, isOutput=False)
    wkv = nc.declare_dram_parameter("wkv", [P, 2, KVW], bf, isOutput=False)
    wo = nc.declare_dram_parameter("wo", [P, 2, D], bf, isOutput=False)
    bo_p = nc.declare_dram_parameter("bo", [1, D], bf, isOutput=False)
    nullkv = nc.declare_dram_parameter("nullkv", [P, KVW], bf, isOutput=False)
    ident_p = nc.declare_dram_parameter("ident", [P, P], bf, isOutput=False)
    idxs_p = nc.declare_dram_parameter("idxs", [P, idxcols], mybir.dt.int16, isOutput=False)
    masks_p = nc.declare_dram_parameter("masks", [P, mcols], f32, isOutput=False)
    out_p = nc.declare_dram_parameter("out", [NT * P, D], f32, isOutput=True)

    kv_dram = nc.dram_tensor("kv_scratch", [NKV, KVW], bf)
    warm_dram = nc.dram_tensor("warm_scratch", [NKV, KVW], bf)

    with tile.TileContext(nc) as tc:
        with (
            tc.tile_pool(name="singles", bufs=1) as singles,
            tc.tile_pool(name="xchunk", bufs=2) as xchunk,
            tc.tile_pool(name="kvstage", bufs=2) as kvstage,
            tc.tile_pool(name="kvgp", bufs=4) as kvgp,
            tc.tile_pool(name="small", bufs=4) as small,
            tc.tile_pool(name="work", bufs=2) as work,
            tc.tile_pool(name="psA", bufs=2, space="PSUM") as psA,
            tc.tile_pool(name="psT", bufs=2, space="PSUM") as psT,
            tc.tile_pool(name="psF", bufs=2, space="PSUM") as psF,
        ):
            # ---------- constants ----------
            wq_sb = singles.tile([P, 2, HD], bf)
            nc.sync.dma_start(out=wq_sb[:], in_=wq[:])
            wkv_sb = singles.tile([P, 2, KVW], bf)
            nc.sync.dma_start(out=wkv_sb[:], in_=wkv[:])
            wo_sb = singles.tile([P, 2, D], bf)
            nc.sync.dma_start(out=wo_sb[:], in_=wo[:])
            bo_sb = singles.tile([1, D], bf)
            nc.sync.dma_start(out=bo_sb[:], in_=bo_p[:])
            nullkv_sb = singles.tile([P, KVW], bf)
            nc.sync.dma_start(out=nullkv_sb[:], in_=nullkv[:])
            ident_sb = singles.tile([P, P], bf)
            nc.sync.dma_start(out=ident_sb[:], in_=ident_p[:])
            idx_sb = singles.tile([P, idxcols], mybir.dt.int16)
            idx_dma = nc.sync.dma_start(out=idx_sb[:], in_=idxs_p[:])
            mask_sb = singles.tile([P, mcols], f32)
            nc.sync.dma_start(out=mask_sb[:], in_=masks_p[:])
            ones1 = singles.tile([1, P], bf)
            nc.vector.memset(ones1[:], 1.0)

            # warmup: force the Q7 dma_gather library load + SWDGE path
            # setup before the first real gather (otherwise ~8us stall).
            warm = small.tile([P, 1, KVW], bf, tag="warm")
            for q in range(2):
                gw = nc.gpsimd.dma_gather(
                    warm[:], warm_dram[:], idx_sb[:, 0:8],
                    num_idxs=P, num_idxs_reg=P, elem_size=KVW,
                    single_packet=False, queue_num=q)
                bass_rust.add_dep_helper(gw.ins, idx_dma.ins,
                                         reason="warmup gather reads idx blob")

            # ---------- phase B: q tiles (small, runs first) ----------
            xp_sb = singles.tile([P, 2, NT * P], bf)
            nc.sync.dma_start(out=xp_sb[:], in_=xpT[:])
            q_sb = singles.tile([P, NT, HD], bf)
            for t in range(NT):
                psq = psF.tile([P, HD], f32, space="PSUM", tag="psF")
                nc.tensor.matmul(
                    out=psq[:], lhsT=xp_sb[:, 0, t * P:(t + 1) * P],
                    rhs=wq_sb[:, 0, :], start=True, stop=False)
                nc.tensor.matmul(
                    out=psq[:], lhsT=xp_sb[:, 1, t * P:(t + 1) * P],
                    rhs=wq_sb[:, 1, :], start=False, stop=True)
                nc.scalar.copy(out=q_sb[:, t, :], in_=psq[:])

            # ---------- phase A: kv table (streamed x chunks) ----------
            # 8 write groups of 10 tiles each; x chunks loaded just-in-time
            # so the sync queue never blocks on a far-future dependency.
            kv_writes = []
            xc_tiles = {}

            def load_xc(c):
                if c >= NWG or c in xc_tiles:
                    return
                w = min(XCH, KV_TILES - c * XCH)
                xc = xchunk.tile([P, XCH, 2, P], bf, tag="xc")
                nc.sync.dma_start(out=xc[:, 0:w, :, :],
                                  in_=xT[:, c * XCH:c * XCH + w, :, :])
                xc_tiles[c] = xc

            load_xc(0)
            load_xc(1)
            gidx = 0
            for g in range(NWG):
                g0 = g * XCH
                nb_g = min(XCH, KV_TILES - g0)
                st = kvstage.tile([P, XCH, KVW], bf, tag="kvstage")
                xc = xc_tiles[g]
                i = 0
                while i < nb_g:
                    nb = min(2, nb_g - i)
                    ps = psA.tile([P, 2, KVW], f32, space="PSUM", tag="psA")
                    for j in range(nb):
                        nc.tensor.matmul(
                            out=ps[:, j, :], lhsT=xc[:, i + j, 0, :],
                            rhs=wkv_sb[:, 0, :], start=True, stop=False)
                        nc.tensor.matmul(
                            out=ps[:, j, :], lhsT=xc[:, i + j, 1, :],
                            rhs=wkv_sb[:, 1, :], start=False, stop=True)
                    if gidx % 2 == 0:
                        nc.scalar.copy(out=st[:, i:i + nb, :], in_=ps[:, 0:nb, :])
                    else:
                        nc.vector.tensor_copy(out=st[:, i:i + nb, :],
                                              in_=ps[:, 0:nb, :])
                    i += nb
                    gidx += 1
                dst = (kv_dram[g0 * P:(g0 + nb_g) * P, :]
                       .rearrange("(j p) c -> p j c", j=nb_g))
                kv_writes.append(
                    nc.sync.dma_start(out=dst, in_=st[:, 0:nb_g, :]))
                load_xc(g + 2)
            assert len(kv_writes) == NWG

            # ---------- phase C: attention per tile ----------
            io = 0
            mo = 0
            for t in range(len(a_ts)):
                a = a_ts[t]
                aa = a + 1
                kv_g = kvgp.tile([P, aa, KVW], bf, tag="kvg")
                nc.scalar.copy(out=kv_g[:, 0, :], in_=nullkv_sb[:])
                chunklist = _chunk_cuts(a, t == 0)
                for ci, (c0, c1) in enumerate(chunklist):
                    gi = nc.gpsimd.dma_gather(
                        kv_g[:, 1 + c0:1 + c1, :], kv_dram[:],
                        idx_sb[:, io + 8 * c0:io + 8 * c1],
                        num_idxs=P * (c1 - c0), num_idxs_reg=P * (c1 - c0),
                        elem_size=KVW, single_packet=False,
                        queue_num=(t + ci) % 2)
                    # Tile's auto-dep tracking misses dma_gather's *input*
                    # APs (idx tile + DRAM source); add edges explicitly.
                    # Edge to EVERY write group up to this chunk's max row
                    # (DMA completion across queues is not FIFO).
                    bass_rust.add_dep_helper(gi.ins, idx_dma.ins,
                                             reason="gather reads idx blob")
                    dep = dep_groups[t][ci]
                    for wgi in range(dep + 1):
                        bass_rust.add_dep_helper(gi.ins, kv_writes[wgi].ins,
                                                 reason="gather reads kv prefix")

                # q.k multiply (bf16 2x) over the whole tile, then one
                # 6-level halving tree over dh -> sim in k[...,0] stripes
                k4 = kv_g[:, :, 0:HD].rearrange("p a (h d) -> p a h d", d=DH)
                qb = (q_sb[:, t:t + 1, :]
                      .rearrange("p o (h d) -> p o h d", d=DH)
                      .broadcast_to([P, aa, H, DH]))
                nc.vector.tensor_tensor(out=k4, in0=k4, in1=qb, op=mult)
                w = DH
                while w > 1:
                    h2 = w // 2
                    nc.vector.tensor_tensor(
                        out=k4[:, :, :, 0:h2], in0=k4[:, :, :, 0:h2],
                        in1=k4[:, :, :, h2:w], op=add)
                    w = h2

                sim = kv_g[:, :, 0:HD:DH]          # [P, aa, H] strided
                exp_s = small.tile([P, aa, H], f32, tag="exp")
                nc.scalar.activation(
                    out=exp_s[:], in_=sim,
                    func=mybir.ActivationFunctionType.Exp)
                # w = mask * exp (bf16 out); unnormalized weights
                wts = small.tile([P, aa, H], bf, tag="wts")
                mb = (mask_sb[:, mo:mo + aa]
                      .rearrange("p (a o) -> p a o", o=1)
                      .broadcast_to([P, aa, H]))
                nc.vector.tensor_tensor(out=wts[:], in0=exp_s[:], in1=mb, op=mult)
                denom = small.tile([P, H], f32, tag="denom")
                nc.vector.tensor_reduce(
                    out=denom[:], in_=wts[:].rearrange("p a h -> p h a"),
                    axis=mybir.AxisListType.X, op=add)
                recip = small.tile([P, H], f32, tag="recip")
                nc.vector.reciprocal(out=recip[:], in_=denom[:])

                # v half is (dh, h)-interleaved: broadcast wts over the
                # *middle* dh axis (stride-0 middle is supported on DVE)
                v4 = kv_g[:, :, HD:KVW].rearrange("p a (d h) -> p a d h", h=H)
                wb = (wts[:].rearrange("p a (o h) -> p a o h", o=1)
                      .broadcast_to([P, aa, DH, H]))
                nc.vector.tensor_tensor(out=v4, in0=v4, in1=wb, op=mult)
                vflat = kv_g[:, :, HD:KVW]          # [P, aa, 256] view
                w = aa
                while w > 1:
                    h2 = w // 2
                    nc.vector.tensor_tensor(
                        out=vflat[:, 0:h2, :], in0=vflat[:, 0:h2, :],
                        in1=vflat[:, h2:2 * h2, :], op=add)
                    if w % 2 == 1:
                        nc.vector.tensor_tensor(
                            out=vflat[:, 0:1, :], in0=vflat[:, 0:1, :],
                            in1=vflat[:, w - 1:w, :], op=add)
                    w = h2
                # normalize the 256-wide sum by 1/denom (broadcast over dh)
                vs = kv_g[:, 0:1, HD:KVW].rearrange("p o (d h) -> p (o d) h", h=H)
                rb = (recip[:].rearrange("p (o h) -> p o h", o=1)
                      .broadcast_to([P, DH, H]))
                nc.vector.tensor_tensor(out=vs, in0=vs, in1=rb, op=mult)

                out_attn = kv_g[:, 0, HD:KVW]      # [P, 256] bf16 (dh,h)
                outT = work.tile([P, 2, P], bf, tag="outT")
                for j in range(2):
                    pst = psT.tile([P, P], bf, space="PSUM", tag="psT")
                    nc.tensor.transpose(
                        out=pst[:], in_=out_attn[:, j * P:(j + 1) * P],
                        identity=ident_sb[:])
                    nc.scalar.copy(out=outT[:, j, :], in_=pst[:])

                psf = psF.tile([P, D], f32, space="PSUM", tag="psF")
                nc.tensor.matmul(out=psf[:], lhsT=ones1[0:1, :], rhs=bo_sb[0:1, :],
                                 start=True, stop=False)
                nc.tensor.matmul(out=psf[:], lhsT=outT[:, 0, :], rhs=wo_sb[:, 0, :],
                                 start=False, stop=False)
                nc.tensor.matmul(out=psf[:], lhsT=outT[:, 1, :], rhs=wo_sb[:, 1, :],
                                 start=False, stop=True)
                outf = small.tile([P, D], f32, tag="outf")
                nc.scalar.copy(out=outf[:], in_=psf[:])
                nc.sync.dma_start(out=out_p[t * P:(t + 1) * P, :], in_=outf[:])

                io += 8 * a
                mo += aa

    nc.finalize()
    return nc


def _prep(x, adj, msk, Wq, Wkv, Wo, bo, null_k, null_v):
    """All host-side numpy prep. Returns (a_ts, dep_groups, in_maps, order)."""
    deg = msk.sum(1).astype(np.int64)
    order = np.concatenate([
        np.full(NPAD - N, -1, dtype=np.int64),
        np.argsort(deg, kind="stable"),
    ])

    a_by_group = []
    for g in range(NT):
        grp = order[g * GROUP:(g + 1) * GROUP]
        real = grp[grp >= 0]
        mx = int(deg[real].max()) if real.size else 0
        a_by_group.append(max(mx, 1))
    # emission order: smallest group first (fast pipeline fill), then
    # descending sizes, ending with small tiles (short drain tail).
    group_order = [0] + list(range(NT - 1, 0, -1))
    a_ts = [a_by_group[g] for g in group_order]

    # compact each node's neighbor list: valid entries first, and sort the
    # valid entries ascending by source row so low slot-columns only touch
    # low kv-table rows (enables gather/phase-A overlap).
    big = np.where(msk, adj, np.int64(1 << 40))
    sortcols = np.argsort(big, axis=1, kind="stable")
    comp = np.take_along_axis(adj, sortcols, axis=1).astype(np.int16)

    # permute v columns of Wkv (and null_v) from (h, dh) to (dh, h) order;
    # permute Wo rows to match.
    vperm = (np.arange(HD).reshape(DH, H) * 0
             + np.arange(H)[None, :] * DH
             + np.arange(DH)[:, None]).reshape(-1)   # (d,h) -> h*DH+d
    Wkv2 = np.concatenate([Wkv[:, :HD], Wkv[:, HD:][:, vperm]], axis=1)
    Wo2 = Wo[vperm, :]
    nv2 = null_v.T.reshape(-1)                        # (d,h) flat
    scale = DH ** -0.5

    wq_h = np.ascontiguousarray(
        (Wq * scale).reshape(2, P, HD).transpose(1, 0, 2)).astype(BF)
    wkv_h = np.ascontiguousarray(
        Wkv2.reshape(2, P, KVW).transpose(1, 0, 2)).astype(BF)
    wo_h = np.ascontiguousarray(
        Wo2.reshape(2, P, D).transpose(1, 0, 2)).astype(BF)
    bo_h = bo.reshape(1, D).astype(BF)
    nullkv_h = np.tile(
        np.concatenate([null_k.reshape(-1), nv2]).reshape(1, KVW),
        (P, 1)).astype(BF)
    ident_h = np.eye(P, dtype=np.float32).astype(BF)

    xpad = np.zeros((NKV, D), np.float32)
    xpad[:N] = x
    # chunk-major xT: [P, KV_TILES, 2, P]; tile i bank j = x.T rows
    # j*128:(j+1)*128, cols i*128:(i+1)*128
    xT_h = np.ascontiguousarray(
        xpad.T.reshape(2, P, KV_TILES, P).transpose(1, 2, 0, 3)).astype(BF)

    in_maps = []
    chunk_maxes = None
    for c in range(NCORES):
        xp = np.zeros((NT * P, D), np.float32)
        flats = []
        mblocks = []
        cmaxes = []
        for t, g in enumerate(group_order):
            a = a_ts[t]
            nodes = order[g * GROUP + c * P: g * GROUP + (c + 1) * P]
            nn = np.maximum(nodes, 0)
            xp[t * P:(t + 1) * P][nodes >= 0] = x[nodes[nodes >= 0]]
            valid = (np.arange(a)[None, :] < deg[nn][:, None]) & (nodes >= 0)[:, None]
            blk = np.where(valid, comp[nn, :a], 0).astype(np.int16)  # [128, a]
            flats.append(blk.T.reshape(-1))                          # i = col*128+p
            m = np.zeros((P, 1 + a), np.float32)
            m[:, 0] = 1.0
            m[:, 1:] = valid
            mblocks.append(m)
            cmaxes.append([int(blk[:, c0:c1].max())
                           for c0, c1 in _chunk_cuts(a, t == 0)])
        flat = np.concatenate(flats)
        idx_h = np.ascontiguousarray(
            np.tile(flat.reshape(-1, 16).T, (8, 1))).astype(np.int16)
        mask_h = np.ascontiguousarray(np.concatenate(mblocks, axis=1))
        xpT_h = np.ascontiguousarray(
            xp.T.reshape(2, P, NT * P).transpose(1, 0, 2)).astype(BF)
        in_maps.append({
            "xT": xT_h, "xpT": xpT_h, "wq": wq_h, "wkv": wkv_h, "wo": wo_h,
            "bo": bo_h, "nullkv": nullkv_h, "ident": ident_h,
            "idxs": idx_h, "masks": mask_h,
        })
        if chunk_maxes is None:
            chunk_maxes = cmaxes
        else:
            chunk_maxes = [[max(a1, b1) for a1, b1 in zip(ra, rb)]
                           for ra, rb in zip(chunk_maxes, cmaxes)]

    # chunk max row -> kv write group (10 tiles = 1280 rows per group)
    dep_groups = tuple(
        tuple(min(m // (XCH * P), NWG - 1) for m in row)
        for row in chunk_maxes)
    return a_ts, dep_groups, in_maps, order


def kernel(x, adj_kv_indices, mask, Wq, Wkv, Wo, bo, null_k, null_v):
    global LAST_EXEC_NS
    x = np.asarray(x, dtype=np.float32)[0]
    adj = np.asarray(adj_kv_indices)[0].astype(np.int64)
    msk = np.asarray(mask)[0].astype(bool)
    Wq = np.asarray(Wq, np.float32)
    Wkv = np.asarray(Wkv, np.float32)
    Wo = np.asarray(Wo, np.float32)
    bo = np.asarray(bo, np.float32)
    null_k = np.asarray(null_k, np.float32)
    null_v = np.asarray(null_v, np.float32)

    a_ts, dep_groups, in_maps, order = _prep(
        x, adj, msk, Wq, Wkv, Wo, bo, null_k, null_v)
    nc = _build(tuple(a_ts), dep_groups)
    res = run_bass_kernel_spmd(
        nc, in_maps, core_ids=list(range(NCORES)),
        trace=bool(os.environ.get("KERNEL_TRACE")))
    LAST_EXEC_NS = res.exec_time_ns

    group_order = [0] + list(range(NT - 1, 0, -1))
    out_full = np.zeros((N, D), np.float32)
    for c in range(NCORES):
        o = np.asarray(res.results[c]["out"])
        for t, g in enumerate(group_order):
            nodes = order[g * GROUP + c * P: g * GROUP + (c + 1) * P]
            sel = nodes >= 0
            out_full[nodes[sel]] = o[t * P:(t + 1) * P][sel]
    return out_full.reshape(1, N, D)
